# revision 37
# baseline (speedup 1.0000x reference)
"""Trainium2 Bass kernel for nn_ConvEnhanced (conv+sigmoid mean / quantum sin^2 mean).

Math:
  classical = mean(sigmoid(conv2d(x, W) + b))           over [32,64,382,382]
  quantum   = mean(win3x3(sin^2(pi*x/2))) / 9           over [32,3,382,382]
  out = 0.5*classical + 0.5*quantum

Strategy (8 cores, batch-sharded, 4 images/core):
  - Classical: conv as matmul with dual block-diagonal weights.
    lhsT [54,128]: rows (dy,i,c,dx)-indexed patch rows for an image pair
    i in {0,1}; cols = 2x64 out-chans. Two weight blocks live at PE rows
    0-53 and 64-117 simultaneously (tile_position row 0/64), so no
    LDWEIGHTS churn between matmuls.
    rhs im2col tiles are loaded by gpsimd (SWDGE) DMAs straight from the
    f32 input with an in-flight cast to bf16; per-partition reads are
    contiguous runs (full 384-wide rows; the (dy,dx) shift only moves the
    start offset), so phase 1 starts immediately.
    Sigmoid+bias+row-sum fused in one ACT op per 4 matmuls (accum_out).
  - Quantum: weighted sum is separable: sum_{i,j} wh(i)*ww(j)*s[i,j].
    On-chip: s = sin(pi/2 * m)^2, m = x - 2*int(x*0.5) (range reduction,
    valid under trunc or RNE cast semantics), then PE matvec wh^T @ s ->
    [1,384] accumulated in PSUM; host applies ww.
  - Host combines per-core partial sums (stats [128,191] f32, qv [1,384]).
"""

import math
from contextlib import ExitStack

import numpy as np

# ---- problem constants (hardcoded) ----
B, C, H, W_ = 32, 3, 384, 384
OC, KK = 64, 3
OH = OW = H - KK + 1  # 382
NCORES = 8
IPC = B // NCORES          # images per core = 4
ICC = IPC * C              # (img, ch) tiles per core = 12
IMG_CH = H * W_            # 147456 elements per (img, ch)
XPAD = 768                 # input tail pad (dx-overrun on last rows)
RC = 32                    # output rows per im2col DMA round
NGROUPS = (2 * OH) // 4    # 191 ACT groups per core (764 matmuls / 4)

_CACHE = {}
LAST_RESULTS = None  # BassKernelResults of the most recent run (for test.py)


def _build():
    import concourse.bacc as bacc
    import concourse.bass as bass
    import concourse.tile as tile
    from concourse import mybir
    from concourse.tile import add_dep_helper

    f32 = mybir.dt.float32
    bf16 = mybir.dt.bfloat16
    i32 = mybir.dt.int32
    Act = mybir.ActivationFunctionType
    Alu = mybir.AluOpType

    nc = bacc.Bacc("TRN2", target_bir_lowering=False, debug=False,
                   num_devices=NCORES)

    x_in = nc.dram_tensor("x", [ICC * IMG_CH + XPAD], f32,
                          kind="ExternalInput")
    w_in = nc.dram_tensor("wmat", [128, 128], bf16, kind="ExternalInput")
    b_in = nc.dram_tensor("bvec", [128, 1], f32, kind="ExternalInput")
    wh_in = nc.dram_tensor("whm", [128, 3], bf16, kind="ExternalInput")
    st_o = nc.dram_tensor("csum", [1, 512], f32, kind="ExternalOutput")
    qv_o = nc.dram_tensor("qv", [1, 384], f32, kind="ExternalOutput")
    x_t = x_in.ap().tensor

    with tile.TileContext(nc) as tc, ExitStack() as ctx:
        singles = ctx.enter_context(tc.tile_pool(name="singles", bufs=1))

        w_sb = singles.tile([128, 128], bf16)
        nc.sync.dma_start(w_sb[:], w_in.ap())
        b_sb = singles.tile([128, 1], f32)
        nc.sync.dma_start(b_sb[:], b_in.ap())
        wh_sb = singles.tile([128, 3], bf16)
        nc.sync.dma_start(wh_sb[:], wh_in.ap())
        qacc = singles.tile([1, 384], f32)
        zb = singles.tile([128, 1], f32)
        nc.vector.memset(zb[:], 0.0)
        ones = singles.tile([128, 1], bf16)
        nc.vector.memset(ones[:], 1.0)
        csb = singles.tile([1, 512], f32)

        first_sin = None
        last_sig = None

        p0 = ctx.enter_context(tc.tile_pool(name="p0", bufs=2))
        xp = ctx.enter_context(tc.tile_pool(name="xp", bufs=2))
        mtp = ctx.enter_context(tc.tile_pool(name="mtp", bufs=6))
        rp = ctx.enter_context(tc.tile_pool(name="rhs", bufs=2))
        sgp = ctx.enter_context(tc.tile_pool(name="sgp", bufs=5))
        pp = ctx.enter_context(tc.tile_pool(name="cpsum", bufs=2, space="PSUM"))
        accp = ctx.enter_context(tc.tile_pool(name="accp", bufs=1, space="PSUM"))

        # ---------------- phase 1: conv + sigmoid + PE row-sums -------------
        # Groups of 3 matmuls -> one Sigmoid ACT op (bf16 out to SBUF) ->
        # ones-matvec on PE accumulating column sums into a single PSUM row
        # (cacc) held across the whole phase.
        cacc = accp.tile([1, 512], f32)
        NMM = 2 * OH            # 764
        GTOT = (NMM + 2) // 3   # 255 groups (last has 2 matmuls)
        n_chunks_total = 0
        for gi in range(GTOT):
            gn = min(3, NMM - gi * 3)
            fd = gn * 382
            n_chunks_total += (fd + 511) // 512
        g = 0
        mm_k = 0
        chunk_i = 0
        psum = None
        nround = 0
        NU = 6
        mts = []
        pending = []  # sigmoid tiles whose ones-matvecs haven't been emitted

        def emit_ones(sg, gn):
            nonlocal chunk_i
            flat = sg[:].rearrange("p a b -> p (a b)")
            fd = gn * 382
            c0 = 0
            while c0 < fd:
                cw = min(512, fd - c0)
                nc.tensor.matmul(
                    cacc[0:1, 0:cw],
                    ones[:, 0:1],
                    flat[:, c0:c0 + cw],
                    start=(chunk_i == 0),
                    stop=(chunk_i == n_chunks_total - 1))
                chunk_i += 1
                c0 += cw

        def flush_group(gn):
            nonlocal g, last_sig
            act_in = psum[:].rearrange(
                "p (k c) -> p k c", k=3)[:, 0:gn, 0:382]
            sg = sgp.tile([128, 3, 382], bf16, tag="sg")
            ins = nc.scalar.activation(
                sg[:, 0:gn, :], act_in, Act.Sigmoid,
                bias=b_sb[:, 0:1], scale=1.0)
            last_sig = ins
            # lag the PE reduction 2 groups behind so PE (FIFO) never
            # waits on this group's ACT
            pending.append((sg, gn))
            if len(pending) > 2:
                emit_ones(*pending.pop(0))
            g += 1

        for r0 in range(0, OH, RC):
            rc = min(RC, OH - r0)
            rt = rp.tile([128, rc * 384], bf16, tag="rt")
            # 6 SWDGE DMAs (2 blocks x 3 dy), casting f32 -> bf16 in
            # flight: partition q = 64b+18dy+9i+3c+dx reads a contiguous
            # rc*384 run of image (2b+i) channel c from row r0+dy, col dx.
            # Runs pair up in traversal order: dest (18, F) <-> src (6,3,F).
            for blk in (0, 1):
                for dy in range(3):
                    dest = rt[64 * blk + 18 * dy:64 * blk + 18 * dy + 18, :]
                    src = bass.AP(
                        tensor=x_t,
                        offset=blk * 6 * IMG_CH + (r0 + dy) * 384,
                        ap=[[IMG_CH, 6], [1, 3], [1, rc * 384]])
                    nc.gpsimd.dma_start(dest, src)
            for blk in (0, 1):
                bp = 64 * blk
                for r in range(rc):
                    if mm_k == 0:
                        psum = pp.tile([128, 1536], f32, tag="ps")
                    nc.tensor.matmul(
                        psum[:, 512 * mm_k:512 * mm_k + 382],
                        w_sb[bp:bp + 54, :],
                        rt[bp:bp + 54, r * 384:r * 384 + 382],
                        start=True, stop=True)
                    mm_k += 1
                    if mm_k == 3:
                        flush_group(3)
                        mm_k = 0
            # interleave quantum input prep (DMA + DVE range reduction)
            # into the round stream so it's ready long before the tail sins
            if nround < NU:
                u = nround
                xt = xp.tile([128, 2304], f32, tag="xt")
                nc.sync.dma_start(
                    xt[:],
                    x_in.ap()[u * 2 * IMG_CH:(u + 1) * 2 * IMG_CH].rearrange(
                        "(p f) -> p f", p=128))
                # range reduction: m = x - 2*int(x*0.5)
                ri = p0.tile([128, 2304], i32, tag="ri")
                nc.vector.tensor_scalar(ri[:], xt[:], 0.5, None, Alu.mult)
                mt = mtp.tile([128, 2304], f32, tag="mt")
                nc.vector.scalar_tensor_tensor(
                    mt[:], ri[:], -2.0, xt[:], Alu.mult, Alu.add)
                mts.append(mt)
            nround += 1
        if mm_k > 0:
            flush_group(mm_k)
            mm_k = 0
        while pending:
            emit_ones(*pending.pop(0))
        assert g == GTOT and chunk_i == n_chunks_total
        nc.vector.tensor_copy(csb[:], cacc[:, :])
        nc.sync.dma_start(st_o.ap(), csb[:])

        # ---------------- phase 2 (tail): quantum sins + reductions ---------
        # ACT sins run after the last sigmoid (single table-set switch);
        # bf16 squares (DVE 2x mode) and wh-matvecs pipeline behind them,
        # accumulating into one PSUM row (conv rotation is finished).
        qp = pp.tile([1, 384], f32, tag="ps")
        for u in range(NU):
            st_t = p0.tile([128, 2304], bf16, tag="st")
            ins = nc.scalar.activation(st_t[:], mts[u][:], Act.Sin,
                                       bias=zb[:, 0:1], scale=math.pi / 2)
            if first_sin is None:
                first_sin = ins
            qt = p0.tile([128, 2304], bf16, tag="qt")
            nc.vector.tensor_mul(qt[:], st_t[:], st_t[:])
            for t in range(6):
                nc.tensor.matmul(
                    qp[:, :],
                    wh_sb[:, t % 3:t % 3 + 1],
                    qt[:, 384 * t:384 * (t + 1)],
                    start=(u == 0 and t == 0),
                    stop=(u == NU - 1 and t == 5))
        nc.vector.tensor_copy(qacc[:], qp[:, :])
        nc.sync.dma_start(qv_o.ap(), qacc[:])

        # keep ACT ops phase-ordered (one table-set switch, not many)
        if first_sin is not None and last_sig is not None:
            add_dep_helper(first_sin.ins, last_sig.ins,
                           reason="sigmoid table-set before sin table-set")

    nc.compile()
    return nc


def _prep_host(W, b):
    # lhsT row order within each 64-block: q = 18*dy + 9*i + 3*c + dx
    wmat = np.zeros((128, 128), dtype=np.float32)
    for base in (0, 64):
        for dy in range(3):
            for i in range(2):
                for c in range(3):
                    for dx in range(3):
                        q = 18 * dy + 9 * i + 3 * c + dx
                        wmat[base + q, 64 * i:64 * i + OC] = W[:, c, dy, dx]
    import ml_dtypes
    wmat = wmat.astype(ml_dtypes.bfloat16)
    bvec = np.concatenate([b, b]).reshape(128, 1).astype(np.float32)
    i = np.arange(H)
    wvec = (np.minimum(i, OH - 1) - np.maximum(i - (KK - 1), 0) + 1)
    whm = wvec.astype(ml_dtypes.bfloat16).reshape(128, 3)
    return wmat, bvec, whm, wvec.astype(np.float64)


def kernel(x, W, b):
    global LAST_RESULTS
    from concourse.bass_utils import run_bass_kernel_spmd

    if "nc" not in _CACHE:
        _CACHE["nc"] = _build()
    nc = _CACHE["nc"]

    x = np.ascontiguousarray(np.asarray(x, dtype=np.float32))
    wmat, bvec, whm, wvec = _prep_host(np.asarray(W, np.float32),
                                       np.asarray(b, np.float32))
    pad = np.zeros(XPAD, np.float32)
    in_maps = []
    for cid in range(NCORES):
        xs = np.concatenate(
            [x[IPC * cid:IPC * (cid + 1)].ravel(), pad])
        in_maps.append({"x": xs, "wmat": wmat, "bvec": bvec, "whm": whm})

    import os
    trace = bool(int(os.environ.get("KERNEL_TRACE", "0")))
    res = run_bass_kernel_spmd(nc, in_maps, core_ids=list(range(NCORES)),
                               trace=trace)
    LAST_RESULTS = res

    cl = 0.0
    qv = np.zeros(384, np.float64)
    for r in res.results:
        cl += r["csum"].astype(np.float64).sum()
        qv += r["qv"][0].astype(np.float64)
    classical_mean = cl / (B * OC * OH * OW)
    quantum_mean = float((qv * wvec).sum()) / (B * C * OH * OW * KK * KK)
    return np.float32(0.5 * classical_mean + 0.5 * quantum_mean)


# revision 40
# speedup vs baseline: 1.0612x; 1.0612x over previous
"""Trainium2 Bass kernel for nn_ConvEnhanced (conv+sigmoid mean / quantum sin^2 mean).

Math:
  classical = mean(sigmoid(conv2d(x, W) + b))           over [32,64,382,382]
  quantum   = mean(win3x3(sin^2(pi*x/2))) / 9           over [32,3,382,382]
  out = 0.5*classical + 0.5*quantum

Strategy (8 cores, batch-sharded, 4 images/core):
  - Classical: conv as matmul with dual block-diagonal weights.
    lhsT [54,128]: rows (dy,i,c,dx)-indexed patch rows for an image pair
    i in {0,1}; cols = 2x64 out-chans. Two weight blocks live at PE rows
    0-53 and 64-117 simultaneously (tile_position row 0/64), so no
    LDWEIGHTS churn between matmuls.
    rhs im2col tiles are loaded by gpsimd (SWDGE) DMAs straight from the
    f32 input with an in-flight cast to bf16; per-partition reads are
    contiguous runs (full 384-wide rows; the (dy,dx) shift only moves the
    start offset), so phase 1 starts immediately.
    Sigmoid+bias+row-sum fused in one ACT op per 4 matmuls (accum_out).
  - Quantum: weighted sum is separable: sum_{i,j} wh(i)*ww(j)*s[i,j].
    On-chip: s = sin(pi/2 * m)^2, m = x - 2*int(x*0.5) (range reduction,
    valid under trunc or RNE cast semantics), then PE matvec wh^T @ s ->
    [1,384] accumulated in PSUM; host applies ww.
  - Host combines per-core partial sums (stats [128,191] f32, qv [1,384]).
"""

import math
from contextlib import ExitStack

import numpy as np

# ---- problem constants (hardcoded) ----
B, C, H, W_ = 32, 3, 384, 384
OC, KK = 64, 3
OH = OW = H - KK + 1  # 382
NCORES = 8
IPC = B // NCORES          # images per core = 4
ICC = IPC * C              # (img, ch) tiles per core = 12
IMG_CH = H * W_            # 147456 elements per (img, ch)
XPAD = 768                 # input tail pad (dx-overrun on last rows)
RC = 32                    # output rows per im2col DMA round
NGROUPS = (2 * OH) // 4    # 191 ACT groups per core (764 matmuls / 4)

_CACHE = {}
LAST_RESULTS = None  # BassKernelResults of the most recent run (for test.py)


def _build():
    import concourse.bacc as bacc
    import concourse.bass as bass
    import concourse.tile as tile
    from concourse import mybir
    from concourse.tile import add_dep_helper

    f32 = mybir.dt.float32
    bf16 = mybir.dt.bfloat16
    i32 = mybir.dt.int32
    Act = mybir.ActivationFunctionType
    Alu = mybir.AluOpType

    nc = bacc.Bacc("TRN2", target_bir_lowering=False, debug=False,
                   num_devices=NCORES)

    x_in = nc.dram_tensor("x", [ICC * IMG_CH + XPAD], f32,
                          kind="ExternalInput")
    w_in = nc.dram_tensor("wmat", [128, 128], bf16, kind="ExternalInput")
    b_in = nc.dram_tensor("bvec", [128, 1], f32, kind="ExternalInput")
    wh_in = nc.dram_tensor("whm", [128, 3], bf16, kind="ExternalInput")
    st_o = nc.dram_tensor("csum", [1, 512], f32, kind="ExternalOutput")
    qv_o = nc.dram_tensor("qv", [1, 384], f32, kind="ExternalOutput")
    x_t = x_in.ap().tensor

    with tile.TileContext(nc) as tc, ExitStack() as ctx:
        singles = ctx.enter_context(tc.tile_pool(name="singles", bufs=1))

        w_sb = singles.tile([128, 128], bf16)
        nc.sync.dma_start(w_sb[:], w_in.ap())
        b_sb = singles.tile([128, 1], f32)
        nc.sync.dma_start(b_sb[:], b_in.ap())
        wh_sb = singles.tile([128, 3], bf16)
        nc.sync.dma_start(wh_sb[:], wh_in.ap())
        qacc = singles.tile([1, 384], f32)
        zb = singles.tile([128, 1], f32)
        nc.vector.memset(zb[:], 0.0)
        ones = singles.tile([128, 1], bf16)
        nc.vector.memset(ones[:], 1.0)
        csb = singles.tile([1, 512], f32)

        first_sin = None
        last_sig = None

        p0 = ctx.enter_context(tc.tile_pool(name="p0", bufs=2))
        xp = ctx.enter_context(tc.tile_pool(name="xp", bufs=2))
        mtp = ctx.enter_context(tc.tile_pool(name="mtp", bufs=6))
        rp = ctx.enter_context(tc.tile_pool(name="rhs", bufs=2))
        sgp = ctx.enter_context(tc.tile_pool(name="sgp", bufs=5))
        pp = ctx.enter_context(tc.tile_pool(name="cpsum", bufs=2, space="PSUM"))
        accp = ctx.enter_context(tc.tile_pool(name="accp", bufs=1, space="PSUM"))

        # ---------------- phase 1: conv + sigmoid + PE row-sums -------------
        # Groups of 3 matmuls -> one Sigmoid ACT op (bf16 out to SBUF) ->
        # ones-matvec on PE accumulating column sums into a single PSUM row
        # (cacc) held across the whole phase.
        cacc = accp.tile([1, 512], f32)
        NMM = 2 * OH            # 764
        GTOT = (NMM + 2) // 3   # 255 groups (last has 2 matmuls)
        # groups are paired into one sg tile (6 matmuls -> 5 ones-chunks)
        n_chunks_total = 0
        rem = NMM
        while rem > 0:
            take = min(6, rem)
            n_chunks_total += (take * 382 + 511) // 512
            rem -= take
        g = 0
        mm_k = 0
        chunk_i = 0
        psum = None
        nround = 0
        NU = 6
        mts = []
        cur_sg = [None]
        pending = []  # (sg, n_mms) whose ones-matvecs haven't been emitted

        def emit_ones(sg, nmm):
            nonlocal chunk_i
            flat = sg[:].rearrange("p a b -> p (a b)")
            fd = nmm * 382
            c0 = 0
            while c0 < fd:
                cw = min(512, fd - c0)
                nc.tensor.matmul(
                    cacc[0:1, 0:cw],
                    ones[:, 0:1],
                    flat[:, c0:c0 + cw],
                    start=(chunk_i == 0),
                    stop=(chunk_i == n_chunks_total - 1))
                chunk_i += 1
                c0 += cw

        def flush_group(gn):
            nonlocal g, last_sig
            act_in = psum[:].rearrange(
                "p (k c) -> p k c", k=3)[:, 0:gn, 0:382]
            h = g % 2
            if h == 0:
                cur_sg[0] = sgp.tile([128, 6, 382], bf16, tag="sg", name="sg")
            sg = cur_sg[0]
            ins = nc.scalar.activation(
                sg[:, 3 * h:3 * h + gn, :], act_in, Act.Sigmoid,
                bias=b_sb[:, 0:1], scale=1.0)
            last_sig = ins
            if h == 1 or g == GTOT - 1:
                # lag the PE reduction one pair behind so PE (FIFO) never
                # waits on this pair's ACT ops
                pending.append((sg, 3 * h + gn))
                if len(pending) > 1:
                    emit_ones(*pending.pop(0))
            g += 1

        # a small first round shortens the pipeline ramp to the first sigmoid
        rounds = [(0, 8)]
        r0 = 8
        while r0 < OH:
            rounds.append((r0, min(RC, OH - r0)))
            r0 += rounds[-1][1]
        for r0, rc in rounds:
            rt = rp.tile([128, rc * 384], bf16, tag="rt")
            # 6 SWDGE DMAs (2 blocks x 3 dy), casting f32 -> bf16 in
            # flight: partition q = 64b+18dy+9i+3c+dx reads a contiguous
            # rc*384 run of image (2b+i) channel c from row r0+dy, col dx.
            # Runs pair up in traversal order: dest (18, F) <-> src (6,3,F).
            for blk in (0, 1):
                for dy in range(3):
                    dest = rt[64 * blk + 18 * dy:64 * blk + 18 * dy + 18, :]
                    src = bass.AP(
                        tensor=x_t,
                        offset=blk * 6 * IMG_CH + (r0 + dy) * 384,
                        ap=[[IMG_CH, 6], [1, 3], [1, rc * 384]])
                    nc.gpsimd.dma_start(dest, src)
            for blk in (0, 1):
                bp = 64 * blk
                for r in range(rc):
                    if mm_k == 0:
                        psum = pp.tile([128, 1536], f32, tag="ps")
                    nc.tensor.matmul(
                        psum[:, 512 * mm_k:512 * mm_k + 382],
                        w_sb[bp:bp + 54, :],
                        rt[bp:bp + 54, r * 384:r * 384 + 382],
                        start=True, stop=True)
                    mm_k += 1
                    if mm_k == 3:
                        flush_group(3)
                        mm_k = 0
            # interleave quantum input prep (DMA + DVE range reduction)
            # into the round stream so it's ready long before the tail sins
            if nround < NU:
                u = nround
                xt = xp.tile([128, 2304], f32, tag="xt")
                nc.sync.dma_start(
                    xt[:],
                    x_in.ap()[u * 2 * IMG_CH:(u + 1) * 2 * IMG_CH].rearrange(
                        "(p f) -> p f", p=128))
                # range reduction: m = x - 2*int(x*0.5)
                ri = p0.tile([128, 2304], i32, tag="ri")
                nc.vector.tensor_scalar(ri[:], xt[:], 0.5, None, Alu.mult)
                mt = mtp.tile([128, 2304], f32, tag="mt")
                nc.vector.scalar_tensor_tensor(
                    mt[:], ri[:], -2.0, xt[:], Alu.mult, Alu.add)
                mts.append(mt)
            nround += 1
        if mm_k > 0:
            flush_group(mm_k)
            mm_k = 0
        while pending:
            emit_ones(*pending.pop(0))
        assert g == GTOT and chunk_i == n_chunks_total
        nc.vector.tensor_copy(csb[:], cacc[:, :])
        nc.sync.dma_start(st_o.ap(), csb[:])

        # ---------------- phase 2 (tail): quantum sins + reductions ---------
        # ACT sins run after the last sigmoid (single table-set switch);
        # bf16 squares (DVE 2x mode) and wh-matvecs pipeline behind them,
        # accumulating into one PSUM row (conv rotation is finished).
        qp = pp.tile([1, 384], f32, tag="ps")
        for u in range(NU):
            st_t = p0.tile([128, 2304], bf16, tag="st")
            ins = nc.scalar.activation(st_t[:], mts[u][:], Act.Sin,
                                       bias=zb[:, 0:1], scale=math.pi / 2)
            if first_sin is None:
                first_sin = ins
            qt = p0.tile([128, 2304], bf16, tag="qt")
            nc.vector.tensor_mul(qt[:], st_t[:], st_t[:])
            for t in range(6):
                nc.tensor.matmul(
                    qp[:, :],
                    wh_sb[:, t % 3:t % 3 + 1],
                    qt[:, 384 * t:384 * (t + 1)],
                    start=(u == 0 and t == 0),
                    stop=(u == NU - 1 and t == 5))
        nc.vector.tensor_copy(qacc[:], qp[:, :])
        nc.sync.dma_start(qv_o.ap(), qacc[:])

        # keep ACT ops phase-ordered (one table-set switch, not many)
        if first_sin is not None and last_sig is not None:
            add_dep_helper(first_sin.ins, last_sig.ins,
                           reason="sigmoid table-set before sin table-set")

    nc.compile()
    return nc


def _prep_host(W, b):
    # lhsT row order within each 64-block: q = 18*dy + 9*i + 3*c + dx
    wmat = np.zeros((128, 128), dtype=np.float32)
    for base in (0, 64):
        for dy in range(3):
            for i in range(2):
                for c in range(3):
                    for dx in range(3):
                        q = 18 * dy + 9 * i + 3 * c + dx
                        wmat[base + q, 64 * i:64 * i + OC] = W[:, c, dy, dx]
    import ml_dtypes
    wmat = wmat.astype(ml_dtypes.bfloat16)
    bvec = np.concatenate([b, b]).reshape(128, 1).astype(np.float32)
    i = np.arange(H)
    wvec = (np.minimum(i, OH - 1) - np.maximum(i - (KK - 1), 0) + 1)
    whm = wvec.astype(ml_dtypes.bfloat16).reshape(128, 3)
    return wmat, bvec, whm, wvec.astype(np.float64)


def kernel(x, W, b):
    global LAST_RESULTS
    from concourse.bass_utils import run_bass_kernel_spmd

    if "nc" not in _CACHE:
        _CACHE["nc"] = _build()
    nc = _CACHE["nc"]

    x = np.ascontiguousarray(np.asarray(x, dtype=np.float32))
    wmat, bvec, whm, wvec = _prep_host(np.asarray(W, np.float32),
                                       np.asarray(b, np.float32))
    pad = np.zeros(XPAD, np.float32)
    in_maps = []
    for cid in range(NCORES):
        xs = np.concatenate(
            [x[IPC * cid:IPC * (cid + 1)].ravel(), pad])
        in_maps.append({"x": xs, "wmat": wmat, "bvec": bvec, "whm": whm})

    import os
    trace = bool(int(os.environ.get("KERNEL_TRACE", "0")))
    res = run_bass_kernel_spmd(nc, in_maps, core_ids=list(range(NCORES)),
                               trace=trace)
    LAST_RESULTS = res

    cl = 0.0
    qv = np.zeros(384, np.float64)
    for r in res.results:
        cl += r["csum"].astype(np.float64).sum()
        qv += r["qv"][0].astype(np.float64)
    classical_mean = cl / (B * OC * OH * OW)
    quantum_mean = float((qv * wvec).sum()) / (B * C * OH * OW * KK * KK)
    return np.float32(0.5 * classical_mean + 0.5 * quantum_mean)
